# revision 21
# baseline (speedup 1.0000x reference)
"""CharEmb kernel for Trainium2 (8 NeuronCores, batch-sharded).

Computation (per word of 32 chars):
  emb = table[ids]                  # [32 chars, 64] gathered
  x[i, j] = emb[i//2, 32*(i%2)+j]   # raw-buffer reshape [64, 32]
  y[f, t] = sum_{i,k} x[i, t+k] * w[f, i, k]   (valid conv, K=3)
  out[f] = max_t y[f, t] + b[f]

Key trick vs naive: a PAIR embedding table pair_tab[v1*101+v2] =
[table[v1] | table[v2]] (128 bf16 = 256B rows) lets one gather
descriptor fetch the char-j rows of TWO adjacent words, halving the
Q7 SWDGE descriptor-generation work (the dominant cost).  Gathers are
issued round-robin on the 4 SWDGE queues so 4 Q7 core-pairs generate
descriptors concurrently.

Device mapping per core (2048 words = 1024 word-pairs, 32768 pair-gathers):
  - gather chunk gc (128 words): pair-idx i -> partition 32s+j holds
    [emb(word 2wp) | emb(word 2wp+1)] where wp = 64gc + 4b + s,
    j = char, b = i//128 block, free offset [b, 256B].
  - conv (per 64-word compute chunk): 6 accumulating K=32 matmuls per
    slot s, row-tiled via tile_position; rhs free dims (b, c2, t)
    pick word = 2wp + c2 and the (h,k) shifted window.
  - maxpool: tensor_reduce(max) over t, alternating Vector/Scalar
    engines per chunk.
"""

import sys
from contextlib import ExitStack

import numpy as np
import ml_dtypes

if "/opt/trn_rl_repo" not in sys.path:
    sys.path.insert(0, "/opt/trn_rl_repo")

import concourse.bass as bass
import concourse.tile as tile
from concourse import bacc, mybir
from concourse.bass_utils import run_bass_kernel_spmd

# Problem constants (hardcoded per spec)
B, S, C = 32, 512, 32
V, E = 101, 64
F, K = 128, 3
T = C - K + 1  # 30 valid conv positions
NCORES = 8
WORDS = (B * S) // NCORES  # 2048 words per core

CWORDS = 64  # words per chunk (one gather + one PSUM tile)
NCCHUNKS = WORDS // CWORDS  # 32
GIDX = CWORDS // 2 * 32  # 1024 pair-indices per chunk
CBLOCKS = GIDX // 128  # 8 blocks of 128 idx per chunk
IDX_COLS = GIDX // 16  # 64 idx columns per chunk
NGCHUNKS = NCCHUNKS  # alias (gather chunk == compute chunk)
GBLOCKS = CBLOCKS

f32 = mybir.dt.float32
bf16 = mybir.dt.bfloat16
i16 = mybir.dt.int16


def build_kernel(num_devices=NCORES):
    nc = bacc.Bacc(
        "TRN2",
        target_bir_lowering=False,
        debug=False,
        enable_asserts=True,
        num_devices=num_devices,
        num_swdge_queues=4,
        dynamic_dma_scratch_size=65536,
    )

    idx_d = nc.dram_tensor("idx", [128, NCCHUNKS * IDX_COLS], i16, kind="ExternalInput")
    tab_d = nc.dram_tensor("tab", [V * V, 2 * E], bf16, kind="ExternalInput")
    w_d = nc.dram_tensor("wmat", [128, 6 * 128], bf16, kind="ExternalInput")
    b_d = nc.dram_tensor("bias", [128, 1], f32, kind="ExternalInput")
    # f-major output: out[f, col]; col = 64c + 16s + 2b + c2
    out_d = nc.dram_tensor("out", [128, WORDS], f32, kind="ExternalOutput")

    IDX_SLICES = 8
    CH_PER_SLICE = NCCHUNKS // IDX_SLICES
    NEG = -1.0e30

    with tile.TileContext(nc) as tc, ExitStack() as ctx:
        const_pool = ctx.enter_context(tc.tile_pool(name="const", bufs=1))
        g_pool = ctx.enter_context(tc.tile_pool(name="gath", bufs=8))
        p_pool = ctx.enter_context(tc.tile_pool(name="psum", bufs=2, space="PSUM"))

        idx_sb = const_pool.tile([128, NCCHUNKS * IDX_COLS], i16)
        w_sb = const_pool.tile([128, 6 * 128], bf16)
        b_sb = const_pool.tile([128, 1], f32)
        obuf = const_pool.tile([128, WORDS], f32)
        # ping-pong scratch for the ACT-copy + DVE max-tree path:
        # per (s, w) 64 cols: [0:30 y | 30:32 pad | 32:48 l1 | 48:56 l2
        #                      | 56:60 l3 | 60:62 l4]
        sbt_tiles = [
            const_pool.tile([128, 4 * 16 * 64], bf16, name=f"sbt{i}")
            for i in range(3)
        ]

        # idx DMA in slices so the first gather doesn't wait for all of it
        scols = CH_PER_SLICE * IDX_COLS
        for d in range(IDX_SLICES):
            nc.sync.dma_start(
                idx_sb[:, d * scols:(d + 1) * scols],
                idx_d.ap()[:, d * scols:(d + 1) * scols],
            )
        nc.sync.dma_start(w_sb[:], w_d.ap())
        nc.sync.dma_start(b_sb[:], b_d.ap())
        for t_ in sbt_tiles:
            # only the 2 pad cols per (s, w) group need -inf; the rest is
            # overwritten every chunk
            pad_v = t_[:].rearrange("f (s w x) -> f s w x", w=16, x=64)
            nc.vector.memset(pad_v[:, :, :, 30:32], NEG)

        nidx_reg = nc.gpsimd.to_reg(GIDX)

        for c in range(NCCHUNKS):
            g = g_pool.tile([128, CBLOCKS * 2 * E], bf16)
            nc.gpsimd.dma_gather(
                out_ap=g[:].rearrange("p (b e) -> p b e", e=2 * E),
                in_ap=tab_d.ap(),
                idxs_ap=idx_sb[:, c * IDX_COLS:(c + 1) * IDX_COLS],
                num_idxs=GIDX,
                num_idxs_reg=nidx_reg,
                elem_size=2 * E,
                single_packet=False,
                queue_num=c % 4,
            )

            # g viewed as [p, b(8), c2(2), e(64)]
            g_r = g[:].rearrange("p (b c2 e) -> p b c2 e", c2=2, e=E)

            # --- conv: 6 accumulating matmuls x 4 row-tiled slots ---
            p = p_pool.tile([128, 4 * 512], f32)
            for hk in range(6):
                h, k = divmod(hk, 3)
                j0 = 32 * h + k
                for s in range(4):
                    out_ap = (
                        p[:, 512 * s:512 * s + CBLOCKS * 2 * T]
                        .rearrange("f (b c2 t) -> f b c2 t", c2=2, t=T)
                    )
                    rhs = g_r[32 * s:32 * s + 32, :, :, j0:j0 + T]
                    lhsT = w_sb[32 * s:32 * s + 32, 128 * hk:128 * hk + 128]
                    nc.tensor.matmul(
                        out_ap,
                        lhsT,
                        rhs,
                        start=(hk == 0),
                        stop=(hk == 5),
                        tile_position=(32 * s, 0),
                        skip_group_check=True,
                    )

            # --- maxpool over t (per word) ---
            p_v = (
                p[:].rearrange("f (s x) -> f s x", x=512)[:, :, 0:CBLOCKS * 2 * T]
                .rearrange("f s (w t) -> f s w t", t=T)
            )
            o_v = (
                obuf[:, c * CWORDS:(c + 1) * CWORDS]
                .rearrange("f (s w) -> f s w", w=2 * CBLOCKS)
            )
            if c % 4 == 0 or c >= NCCHUNKS - 3:
                # direct DVE reduce from PSUM (also for the tail chunks:
                # shorter drain latency than the ACT-copy + tree path)
                nc.vector.tensor_reduce(
                    o_v, p_v, axis=mybir.AxisListType.X, op=mybir.AluOpType.max
                )
            else:
                # ACT copies PSUM->SBUF bf16; DVE does a 2x-mode max tree
                sbt = sbt_tiles[c % 4 - 1]
                s_v = sbt[:].rearrange("f (s w x) -> f s w x", w=16, x=64)
                nc.scalar.copy(s_v[:, :, :, 0:30], p_v)
                nc.vector.tensor_max(
                    s_v[:, :, :, 32:48], s_v[:, :, :, 0:16], s_v[:, :, :, 16:32]
                )
                nc.vector.tensor_max(
                    s_v[:, :, :, 48:56], s_v[:, :, :, 32:40], s_v[:, :, :, 40:48]
                )
                nc.vector.tensor_max(
                    s_v[:, :, :, 56:60], s_v[:, :, :, 48:52], s_v[:, :, :, 52:56]
                )
                nc.vector.tensor_max(
                    s_v[:, :, :, 60:62], s_v[:, :, :, 56:58], s_v[:, :, :, 58:60]
                )
                o_v1 = (
                    obuf[:, c * CWORDS:(c + 1) * CWORDS]
                    .rearrange("f (s w x) -> f s w x", w=2 * CBLOCKS, x=1)
                )
                nc.vector.tensor_max(
                    o_v1, s_v[:, :, :, 60:61], s_v[:, :, :, 61:62]
                )

            # --- per-quarter bias + store (overlap the tail) ---
            if c % 8 == 7:
                q = c // 8
                seg = obuf[:, q * 512:(q + 1) * 512]
                nc.scalar.activation(
                    seg, seg, mybir.ActivationFunctionType.Identity,
                    bias=b_sb[:, 0:1],
                )
                nc.sync.dma_start(out_d.ap()[:, q * 512:(q + 1) * 512], seg)

    nc.compile()
    return nc


def host_prep(char_ids, emb_table, conv_w, conv_b, num_devices=NCORES):
    """Build per-core input maps from full inputs."""
    char_ids = np.asarray(char_ids)
    emb_table = np.asarray(emb_table, dtype=np.float32)
    conv_w = np.asarray(conv_w, dtype=np.float32)
    conv_b = np.asarray(conv_b, dtype=np.float32)

    # pair table: ptab[v1*V+v2] = [table[v1] | table[v2]], bf16
    tab_bf = emb_table.astype(ml_dtypes.bfloat16)
    ptab = np.empty((V * V, 2 * E), dtype=ml_dtypes.bfloat16)
    ptab[:, :E] = np.repeat(tab_bf, V, axis=0)
    ptab[:, E:] = np.tile(tab_bf, (V, 1))

    # stationary weights: wmat[32s+j, 128*(3h+k) + f] = conv_w[f, 2j+h, k]
    wmat = np.zeros((128, 6 * 128), dtype=np.float32)
    for h in range(2):
        for k in range(3):
            hk = 3 * h + k
            w_pf = conv_w[:, h::2, k].T  # [32 j, 128 f]
            wmat[:, 128 * hk:128 * (hk + 1)] = np.tile(w_pf, (4, 1))
    wmat = wmat.astype(ml_dtypes.bfloat16)

    bias = conv_b.reshape(128, 1)

    ids_flat = char_ids.reshape(-1, C).astype(np.int32)  # [16384 words, 32]

    in_maps = []
    for jcore in range(num_devices):
        ids_core = ids_flat[jcore * WORDS:(jcore + 1) * WORDS]  # [2048, 32]
        # pair idx, ordered i = gc*2048 + b*128 + s*32 + j with wp = 64gc+4b+s
        idv = ids_core.reshape(WORDS // 2, 2, C)  # [wp, 2, j]
        pidx = idv[:, 0, :] * V + idv[:, 1, :]  # [1024 wp, 32 j]
        # wp = 64gc + 4b + s -> order (gc, b, s, j)
        pidx = pidx.reshape(NGCHUNKS, GBLOCKS, 4, C)  # [gc, b, s, j]
        flat = pidx.reshape(-1).astype(np.int16)  # i-order
        ncols = flat.size // 16
        wrapped = flat.reshape(ncols, 16).T.copy()  # [16, ncols]
        idx = np.tile(wrapped, (8, 1))  # replicate to 128 partitions
        in_maps.append(
            {
                "idx": np.ascontiguousarray(idx),
                "tab": ptab,
                "wmat": wmat,
                "bias": bias,
            }
        )
    return in_maps


def _ensure_ntff_hook():
    """The agent image's antenv lacks axon_hooks; shim it and install the
    ctypes NTFF profiling hook so trace=True yields HW exec times."""
    import types

    if "antenv.axon_hooks" in sys.modules:
        return
    mod = types.ModuleType("antenv.axon_hooks")
    _hook = [None]
    mod.get_axon_ntff_profile_hook = lambda: _hook[0]
    mod.set_axon_ntff_profile_hook = lambda h: _hook.__setitem__(0, h)
    sys.modules["antenv.axon_hooks"] = mod
    try:
        import antenv

        antenv.axon_hooks = mod
        from trn_agent_boot.trn_boot import _ntff_profile_via_ctypes

        hook = _ntff_profile_via_ctypes("/opt/axon/libaxon_pjrt.so")
        mod.set_axon_ntff_profile_hook(hook)
    except Exception as e:  # degrade to no-trace
        print(f"ntff hook install failed: {e}", file=sys.stderr)


_NC_CACHE = {}


def _get_nc():
    if "nc" not in _NC_CACHE:
        _NC_CACHE["nc"] = build_kernel()
    return _NC_CACHE["nc"]


def unscramble_out(raw):
    """[128 f, 2048 cols], col = 64c+16s+2b+c2 -> [word, 128] word-major.

    word = 64c + 8b + 2s + c2."""
    o = raw.reshape(128, NCCHUNKS, 4, CBLOCKS, 2)  # [f, c, s, b, c2]
    o = o.transpose(1, 3, 2, 4, 0)  # [c, b, s, c2, f]
    # word = 64c + 8b + 2s + c2 -> index order (c, b, s, c2)
    return np.ascontiguousarray(o.reshape(WORDS, 128))


def kernel(char_ids, emb_table, conv_w, conv_b, trace=False):
    if trace:
        _ensure_ntff_hook()
    nc = _get_nc()
    in_maps = host_prep(char_ids, emb_table, conv_w, conv_b)
    res = run_bass_kernel_spmd(
        nc, in_maps, core_ids=list(range(NCORES)), trace=trace
    )
    outs = [unscramble_out(res.results[j]["out"]) for j in range(NCORES)]
    full = np.concatenate(outs, axis=0).reshape(B, S, F).astype(np.float32)
    if trace:
        return full, res
    return full


# revision 23
# speedup vs baseline: 1.0058x; 1.0058x over previous
"""CharEmb kernel for Trainium2 (8 NeuronCores, batch-sharded).

Computation (per word of 32 chars):
  emb = table[ids]                  # [32 chars, 64] gathered
  x[i, j] = emb[i//2, 32*(i%2)+j]   # raw-buffer reshape [64, 32]
  y[f, t] = sum_{i,k} x[i, t+k] * w[f, i, k]   (valid conv, K=3)
  out[f] = max_t y[f, t] + b[f]

Key trick vs naive: a PAIR embedding table pair_tab[v1*101+v2] =
[table[v1] | table[v2]] (128 bf16 = 256B rows) lets one gather
descriptor fetch the char-j rows of TWO adjacent words, halving the
Q7 SWDGE descriptor-generation work (the dominant cost).  Gathers are
issued round-robin on the 4 SWDGE queues so 4 Q7 core-pairs generate
descriptors concurrently.

Device mapping per core (2048 words = 1024 word-pairs, 32768 pair-gathers):
  - gather chunk gc (128 words): pair-idx i -> partition 32s+j holds
    [emb(word 2wp) | emb(word 2wp+1)] where wp = 64gc + 4b + s,
    j = char, b = i//128 block, free offset [b, 256B].
  - conv (per 64-word compute chunk): 6 accumulating K=32 matmuls per
    slot s, row-tiled via tile_position; rhs free dims (b, c2, t)
    pick word = 2wp + c2 and the (h,k) shifted window.
  - maxpool: tensor_reduce(max) over t, alternating Vector/Scalar
    engines per chunk.
"""

import sys
from contextlib import ExitStack

import numpy as np
import ml_dtypes

if "/opt/trn_rl_repo" not in sys.path:
    sys.path.insert(0, "/opt/trn_rl_repo")

import concourse.bass as bass
import concourse.tile as tile
from concourse import bacc, mybir
from concourse.bass_utils import run_bass_kernel_spmd

# Problem constants (hardcoded per spec)
B, S, C = 32, 512, 32
V, E = 101, 64
F, K = 128, 3
T = C - K + 1  # 30 valid conv positions
NCORES = 8
WORDS = (B * S) // NCORES  # 2048 words per core

CWORDS = 64  # words per chunk (one gather + one PSUM tile)
NCCHUNKS = WORDS // CWORDS  # 32
GIDX = CWORDS // 2 * 32  # 1024 pair-indices per chunk
CBLOCKS = GIDX // 128  # 8 blocks of 128 idx per chunk
IDX_COLS = GIDX // 16  # 64 idx columns per chunk
NGCHUNKS = NCCHUNKS  # alias (gather chunk == compute chunk)
GBLOCKS = CBLOCKS

f32 = mybir.dt.float32
bf16 = mybir.dt.bfloat16
i16 = mybir.dt.int16


def build_kernel(num_devices=NCORES):
    nc = bacc.Bacc(
        "TRN2",
        target_bir_lowering=False,
        debug=False,
        enable_asserts=True,
        num_devices=num_devices,
        num_swdge_queues=4,
        dynamic_dma_scratch_size=65536,
    )

    idx_d = nc.dram_tensor("idx", [128, NCCHUNKS * IDX_COLS], i16, kind="ExternalInput")
    tab_d = nc.dram_tensor("tab", [V * V, 2 * E], bf16, kind="ExternalInput")
    w_d = nc.dram_tensor("wmat", [128, 6 * 128], bf16, kind="ExternalInput")
    b_d = nc.dram_tensor("bias", [128, 1], f32, kind="ExternalInput")
    # f-major output: out[f, col]; col = 64c + 16s + 2b + c2
    out_d = nc.dram_tensor("out", [128, WORDS], f32, kind="ExternalOutput")

    IDX_SLICES = 8
    CH_PER_SLICE = NCCHUNKS // IDX_SLICES
    NEG = -1.0e30

    with tile.TileContext(nc) as tc, ExitStack() as ctx:
        const_pool = ctx.enter_context(tc.tile_pool(name="const", bufs=1))
        g_pool = ctx.enter_context(tc.tile_pool(name="gath", bufs=8))
        p_pool = ctx.enter_context(tc.tile_pool(name="psum", bufs=2, space="PSUM"))

        idx_sb = const_pool.tile([128, NCCHUNKS * IDX_COLS], i16)
        w_sb = const_pool.tile([128, 6 * 128], bf16)
        b_sb = const_pool.tile([128, 1], f32)
        obuf = const_pool.tile([128, WORDS], f32)
        # ping-pong scratch for the ACT-copy + DVE max-tree path:
        # per (s, w) 64 cols: [0:30 y | 30:32 pad | 32:48 l1 | 48:56 l2
        #                      | 56:60 l3 | 60:62 l4]
        sbt_tiles = [
            const_pool.tile([128, 4 * 16 * 64], bf16, name=f"sbt{i}")
            for i in range(3)
        ]

        # idx DMA in slices so the first gather doesn't wait for all of it
        scols = CH_PER_SLICE * IDX_COLS
        for d in range(IDX_SLICES):
            nc.sync.dma_start(
                idx_sb[:, d * scols:(d + 1) * scols],
                idx_d.ap()[:, d * scols:(d + 1) * scols],
            )
        nc.sync.dma_start(w_sb[:], w_d.ap())
        nc.sync.dma_start(b_sb[:], b_d.ap())
        for t_ in sbt_tiles:
            # only the 2 pad cols per (s, w) group need -inf; the rest is
            # overwritten every chunk
            pad_v = t_[:].rearrange("f (s w x) -> f s w x", w=16, x=64)
            nc.vector.memset(pad_v[:, :, :, 30:32], NEG)

        nidx_reg = nc.gpsimd.to_reg(GIDX)

        for c in range(NCCHUNKS):
            g = g_pool.tile([128, CBLOCKS * 2 * E], bf16)
            nc.gpsimd.dma_gather(
                out_ap=g[:].rearrange("p (b e) -> p b e", e=2 * E),
                in_ap=tab_d.ap(),
                idxs_ap=idx_sb[:, c * IDX_COLS:(c + 1) * IDX_COLS],
                num_idxs=GIDX,
                num_idxs_reg=nidx_reg,
                elem_size=2 * E,
                single_packet=False,
                queue_num=c % 4,
            )

            # g viewed as [p, b(8), c2(2), e(64)]
            g_r = g[:].rearrange("p (b c2 e) -> p b c2 e", c2=2, e=E)

            # --- conv: 6 accumulating matmuls x 4 row-tiled slots ---
            p = p_pool.tile([128, 4 * 512], f32)
            for hk in range(6):
                h, k = divmod(hk, 3)
                j0 = 32 * h + k
                for s in range(4):
                    out_ap = (
                        p[:, 512 * s:512 * s + CBLOCKS * 2 * T]
                        .rearrange("f (b c2 t) -> f b c2 t", c2=2, t=T)
                    )
                    rhs = g_r[32 * s:32 * s + 32, :, :, j0:j0 + T]
                    lhsT = w_sb[32 * s:32 * s + 32, 128 * hk:128 * hk + 128]
                    nc.tensor.matmul(
                        out_ap,
                        lhsT,
                        rhs,
                        start=(hk == 0),
                        stop=(hk == 5),
                        tile_position=(32 * s, 0),
                        skip_group_check=True,
                    )

            # --- maxpool over t (per word) ---
            p_v = (
                p[:].rearrange("f (s x) -> f s x", x=512)[:, :, 0:CBLOCKS * 2 * T]
                .rearrange("f s (w t) -> f s w t", t=T)
            )
            o_v = (
                obuf[:, c * CWORDS:(c + 1) * CWORDS]
                .rearrange("f (s w) -> f s w", w=2 * CBLOCKS)
            )
            if c % 4 == 0 or c >= NCCHUNKS - 3:
                # direct DVE reduce from PSUM (also for the tail chunks:
                # shorter drain latency than the ACT-copy + tree path),
                # then a tiny per-chunk bias add on ACT
                nc.vector.tensor_reduce(
                    o_v, p_v, axis=mybir.AxisListType.X, op=mybir.AluOpType.max
                )
                seg = obuf[:, c * CWORDS:(c + 1) * CWORDS]
                nc.scalar.add(seg, seg, b_sb[:, 0:1])
            else:
                # ACT copies PSUM->SBUF bf16 with the bias folded in
                # (max(y+b) == max(y)+b); DVE does a 2x-mode max tree
                sbt = sbt_tiles[c % 4 - 1]
                s_v = sbt[:].rearrange("f (s w x) -> f s w x", w=16, x=64)
                nc.scalar.activation(
                    s_v[:, :, :, 0:30], p_v,
                    mybir.ActivationFunctionType.Identity, bias=b_sb[:, 0:1],
                )
                nc.vector.tensor_max(
                    s_v[:, :, :, 32:48], s_v[:, :, :, 0:16], s_v[:, :, :, 16:32]
                )
                nc.vector.tensor_max(
                    s_v[:, :, :, 48:56], s_v[:, :, :, 32:40], s_v[:, :, :, 40:48]
                )
                nc.vector.tensor_max(
                    s_v[:, :, :, 56:60], s_v[:, :, :, 48:52], s_v[:, :, :, 52:56]
                )
                nc.vector.tensor_max(
                    s_v[:, :, :, 60:62], s_v[:, :, :, 56:58], s_v[:, :, :, 58:60]
                )
                o_v1 = (
                    obuf[:, c * CWORDS:(c + 1) * CWORDS]
                    .rearrange("f (s w x) -> f s w x", w=2 * CBLOCKS, x=1)
                )
                nc.vector.tensor_max(
                    o_v1, s_v[:, :, :, 60:61], s_v[:, :, :, 61:62]
                )

            # --- per-quarter store (bias already applied per chunk) ---
            if c % 8 == 7:
                q = c // 8
                nc.sync.dma_start(
                    out_d.ap()[:, q * 512:(q + 1) * 512],
                    obuf[:, q * 512:(q + 1) * 512],
                )

    nc.compile()
    return nc


def host_prep(char_ids, emb_table, conv_w, conv_b, num_devices=NCORES):
    """Build per-core input maps from full inputs."""
    char_ids = np.asarray(char_ids)
    emb_table = np.asarray(emb_table, dtype=np.float32)
    conv_w = np.asarray(conv_w, dtype=np.float32)
    conv_b = np.asarray(conv_b, dtype=np.float32)

    # pair table: ptab[v1*V+v2] = [table[v1] | table[v2]], bf16
    tab_bf = emb_table.astype(ml_dtypes.bfloat16)
    ptab = np.empty((V * V, 2 * E), dtype=ml_dtypes.bfloat16)
    ptab[:, :E] = np.repeat(tab_bf, V, axis=0)
    ptab[:, E:] = np.tile(tab_bf, (V, 1))

    # stationary weights: wmat[32s+j, 128*(3h+k) + f] = conv_w[f, 2j+h, k]
    wmat = np.zeros((128, 6 * 128), dtype=np.float32)
    for h in range(2):
        for k in range(3):
            hk = 3 * h + k
            w_pf = conv_w[:, h::2, k].T  # [32 j, 128 f]
            wmat[:, 128 * hk:128 * (hk + 1)] = np.tile(w_pf, (4, 1))
    wmat = wmat.astype(ml_dtypes.bfloat16)

    bias = conv_b.reshape(128, 1)

    ids_flat = char_ids.reshape(-1, C).astype(np.int32)  # [16384 words, 32]

    in_maps = []
    for jcore in range(num_devices):
        ids_core = ids_flat[jcore * WORDS:(jcore + 1) * WORDS]  # [2048, 32]
        # pair idx, ordered i = gc*2048 + b*128 + s*32 + j with wp = 64gc+4b+s
        idv = ids_core.reshape(WORDS // 2, 2, C)  # [wp, 2, j]
        pidx = idv[:, 0, :] * V + idv[:, 1, :]  # [1024 wp, 32 j]
        # wp = 64gc + 4b + s -> order (gc, b, s, j)
        pidx = pidx.reshape(NGCHUNKS, GBLOCKS, 4, C)  # [gc, b, s, j]
        flat = pidx.reshape(-1).astype(np.int16)  # i-order
        ncols = flat.size // 16
        wrapped = flat.reshape(ncols, 16).T.copy()  # [16, ncols]
        idx = np.tile(wrapped, (8, 1))  # replicate to 128 partitions
        in_maps.append(
            {
                "idx": np.ascontiguousarray(idx),
                "tab": ptab,
                "wmat": wmat,
                "bias": bias,
            }
        )
    return in_maps


def _ensure_ntff_hook():
    """The agent image's antenv lacks axon_hooks; shim it and install the
    ctypes NTFF profiling hook so trace=True yields HW exec times."""
    import types

    if "antenv.axon_hooks" in sys.modules:
        return
    mod = types.ModuleType("antenv.axon_hooks")
    _hook = [None]
    mod.get_axon_ntff_profile_hook = lambda: _hook[0]
    mod.set_axon_ntff_profile_hook = lambda h: _hook.__setitem__(0, h)
    sys.modules["antenv.axon_hooks"] = mod
    try:
        import antenv

        antenv.axon_hooks = mod
        from trn_agent_boot.trn_boot import _ntff_profile_via_ctypes

        hook = _ntff_profile_via_ctypes("/opt/axon/libaxon_pjrt.so")
        mod.set_axon_ntff_profile_hook(hook)
    except Exception as e:  # degrade to no-trace
        print(f"ntff hook install failed: {e}", file=sys.stderr)


_NC_CACHE = {}


def _get_nc():
    if "nc" not in _NC_CACHE:
        _NC_CACHE["nc"] = build_kernel()
    return _NC_CACHE["nc"]


def unscramble_out(raw):
    """[128 f, 2048 cols], col = 64c+16s+2b+c2 -> [word, 128] word-major.

    word = 64c + 8b + 2s + c2."""
    o = raw.reshape(128, NCCHUNKS, 4, CBLOCKS, 2)  # [f, c, s, b, c2]
    o = o.transpose(1, 3, 2, 4, 0)  # [c, b, s, c2, f]
    # word = 64c + 8b + 2s + c2 -> index order (c, b, s, c2)
    return np.ascontiguousarray(o.reshape(WORDS, 128))


def kernel(char_ids, emb_table, conv_w, conv_b, trace=False):
    if trace:
        _ensure_ntff_hook()
    nc = _get_nc()
    in_maps = host_prep(char_ids, emb_table, conv_w, conv_b)
    res = run_bass_kernel_spmd(
        nc, in_maps, core_ids=list(range(NCORES)), trace=trace
    )
    outs = [unscramble_out(res.results[j]["out"]) for j in range(NCORES)]
    full = np.concatenate(outs, axis=0).reshape(B, S, F).astype(np.float32)
    if trace:
        return full, res
    return full


# revision 25
# speedup vs baseline: 1.0157x; 1.0098x over previous
"""CharEmb kernel for Trainium2 (8 NeuronCores, batch-sharded).

Computation (per word of 32 chars):
  emb = table[ids]                  # [32 chars, 64] gathered
  x[i, j] = emb[i//2, 32*(i%2)+j]   # raw-buffer reshape [64, 32]
  y[f, t] = sum_{i,k} x[i, t+k] * w[f, i, k]   (valid conv, K=3)
  out[f] = max_t y[f, t] + b[f]

Key trick vs naive: a PAIR embedding table pair_tab[v1*101+v2] =
[table[v1] | table[v2]] (128 bf16 = 256B rows) lets one gather
descriptor fetch the char-j rows of TWO adjacent words, halving the
Q7 SWDGE descriptor-generation work (the dominant cost).  Gathers are
issued round-robin on the 4 SWDGE queues so 4 Q7 core-pairs generate
descriptors concurrently.

Device mapping per core (2048 words = 1024 word-pairs, 32768 pair-gathers):
  - gather chunk gc (128 words): pair-idx i -> partition 32s+j holds
    [emb(word 2wp) | emb(word 2wp+1)] where wp = 64gc + 4b + s,
    j = char, b = i//128 block, free offset [b, 256B].
  - conv (per 64-word compute chunk): 6 accumulating K=32 matmuls per
    slot s, row-tiled via tile_position; rhs free dims (b, c2, t)
    pick word = 2wp + c2 and the (h,k) shifted window.
  - maxpool: tensor_reduce(max) over t, alternating Vector/Scalar
    engines per chunk.
"""

import sys
from contextlib import ExitStack

import numpy as np
import ml_dtypes

if "/opt/trn_rl_repo" not in sys.path:
    sys.path.insert(0, "/opt/trn_rl_repo")

import concourse.bass as bass
import concourse.tile as tile
from concourse import bacc, mybir
from concourse.bass_utils import run_bass_kernel_spmd

# Problem constants (hardcoded per spec)
B, S, C = 32, 512, 32
V, E = 101, 64
F, K = 128, 3
T = C - K + 1  # 30 valid conv positions
NCORES = 8
WORDS = (B * S) // NCORES  # 2048 words per core

CWORDS = 64  # words per chunk (one gather + one PSUM tile)
NCCHUNKS = WORDS // CWORDS  # 32
GIDX = CWORDS // 2 * 32  # 1024 pair-indices per chunk
CBLOCKS = GIDX // 128  # 8 blocks of 128 idx per chunk
IDX_COLS = GIDX // 16  # 64 idx columns per chunk
NGCHUNKS = NCCHUNKS  # alias (gather chunk == compute chunk)
GBLOCKS = CBLOCKS

f32 = mybir.dt.float32
bf16 = mybir.dt.bfloat16
i16 = mybir.dt.int16


def build_kernel(num_devices=NCORES):
    nc = bacc.Bacc(
        "TRN2",
        target_bir_lowering=False,
        debug=False,
        enable_asserts=True,
        num_devices=num_devices,
        num_swdge_queues=4,
        dynamic_dma_scratch_size=65536,
    )

    idx_d = nc.dram_tensor("idx", [128, NCCHUNKS * IDX_COLS], i16, kind="ExternalInput")
    tab_d = nc.dram_tensor("tab", [V * V, 2 * E], bf16, kind="ExternalInput")
    w_d = nc.dram_tensor("wmat", [128, 6 * 128], bf16, kind="ExternalInput")
    b_d = nc.dram_tensor("bias", [128, 1], f32, kind="ExternalInput")
    # f-major output: out[f, col]; col = 64c + 16s + 2b + c2
    out_d = nc.dram_tensor("out", [128, WORDS], f32, kind="ExternalOutput")

    IDX_SLICES = 8
    CH_PER_SLICE = NCCHUNKS // IDX_SLICES
    NEG = -1.0e30

    with tile.TileContext(nc) as tc, ExitStack() as ctx:
        const_pool = ctx.enter_context(tc.tile_pool(name="const", bufs=1))
        g_pool = ctx.enter_context(tc.tile_pool(name="gath", bufs=12))
        p_pool = ctx.enter_context(tc.tile_pool(name="psum", bufs=2, space="PSUM"))

        idx_sb = const_pool.tile([128, NCCHUNKS * IDX_COLS], i16)
        w_sb = const_pool.tile([128, 6 * 128], bf16)
        b_sb = const_pool.tile([128, 1], f32)
        obuf = const_pool.tile([128, WORDS], f32)
        # ping-pong scratch for the ACT-copy + DVE max-tree path:
        # per (s, w) 64 cols: [0:30 y | 30:32 pad | 32:48 l1 | 48:56 l2
        #                      | 56:60 l3 | 60:62 l4]
        sbt_tiles = [
            const_pool.tile([128, 4 * 16 * 64], bf16, name=f"sbt{i}")
            for i in range(3)
        ]

        # idx DMA in slices so the first gather doesn't wait for all of it
        scols = CH_PER_SLICE * IDX_COLS
        for d in range(IDX_SLICES):
            nc.sync.dma_start(
                idx_sb[:, d * scols:(d + 1) * scols],
                idx_d.ap()[:, d * scols:(d + 1) * scols],
            )
        nc.sync.dma_start(w_sb[:], w_d.ap())
        nc.sync.dma_start(b_sb[:], b_d.ap())
        for t_ in sbt_tiles:
            # only the 2 pad cols per (s, w) group need -inf; the rest is
            # overwritten every chunk
            pad_v = t_[:].rearrange("f (s w x) -> f s w x", w=16, x=64)
            nc.vector.memset(pad_v[:, :, :, 30:32], NEG)

        nidx_reg = nc.gpsimd.to_reg(GIDX)

        for c in range(NCCHUNKS):
            g = g_pool.tile([128, CBLOCKS * 2 * E], bf16)
            nc.gpsimd.dma_gather(
                out_ap=g[:].rearrange("p (b e) -> p b e", e=2 * E),
                in_ap=tab_d.ap(),
                idxs_ap=idx_sb[:, c * IDX_COLS:(c + 1) * IDX_COLS],
                num_idxs=GIDX,
                num_idxs_reg=nidx_reg,
                elem_size=2 * E,
                single_packet=False,
                queue_num=c % 4,
            )

            # g viewed as [p, b(8), c2(2), e(64)]
            g_r = g[:].rearrange("p (b c2 e) -> p b c2 e", c2=2, e=E)

            # --- conv: 6 accumulating matmuls x 4 row-tiled slots ---
            p = p_pool.tile([128, 4 * 512], f32)
            for hk in range(6):
                h, k = divmod(hk, 3)
                j0 = 32 * h + k
                for s in range(4):
                    out_ap = (
                        p[:, 512 * s:512 * s + CBLOCKS * 2 * T]
                        .rearrange("f (b c2 t) -> f b c2 t", c2=2, t=T)
                    )
                    rhs = g_r[32 * s:32 * s + 32, :, :, j0:j0 + T]
                    lhsT = w_sb[32 * s:32 * s + 32, 128 * hk:128 * hk + 128]
                    nc.tensor.matmul(
                        out_ap,
                        lhsT,
                        rhs,
                        start=(hk == 0),
                        stop=(hk == 5),
                        tile_position=(32 * s, 0),
                        skip_group_check=True,
                    )

            # --- maxpool over t (per word) ---
            p_v = (
                p[:].rearrange("f (s x) -> f s x", x=512)[:, :, 0:CBLOCKS * 2 * T]
                .rearrange("f s (w t) -> f s w t", t=T)
            )
            o_v = (
                obuf[:, c * CWORDS:(c + 1) * CWORDS]
                .rearrange("f (s w) -> f s w", w=2 * CBLOCKS)
            )
            if c % 4 == 0 or c >= NCCHUNKS - 3:
                # direct DVE reduce from PSUM (also for the tail chunks:
                # shorter drain latency than the ACT-copy + tree path),
                # then a tiny per-chunk bias add on ACT
                nc.vector.tensor_reduce(
                    o_v, p_v, axis=mybir.AxisListType.X, op=mybir.AluOpType.max
                )
                seg = obuf[:, c * CWORDS:(c + 1) * CWORDS]
                nc.scalar.add(seg, seg, b_sb[:, 0:1])
            else:
                # ACT copies PSUM->SBUF bf16 with the bias folded in
                # (max(y+b) == max(y)+b); DVE does a 2x-mode max tree
                sbt = sbt_tiles[c % 4 - 1]
                s_v = sbt[:].rearrange("f (s w x) -> f s w x", w=16, x=64)
                nc.scalar.activation(
                    s_v[:, :, :, 0:30], p_v,
                    mybir.ActivationFunctionType.Identity, bias=b_sb[:, 0:1],
                )
                nc.vector.tensor_max(
                    s_v[:, :, :, 32:48], s_v[:, :, :, 0:16], s_v[:, :, :, 16:32]
                )
                nc.vector.tensor_max(
                    s_v[:, :, :, 48:56], s_v[:, :, :, 32:40], s_v[:, :, :, 40:48]
                )
                nc.vector.tensor_max(
                    s_v[:, :, :, 56:60], s_v[:, :, :, 48:52], s_v[:, :, :, 52:56]
                )
                nc.vector.tensor_max(
                    s_v[:, :, :, 60:62], s_v[:, :, :, 56:58], s_v[:, :, :, 58:60]
                )
                o_v1 = (
                    obuf[:, c * CWORDS:(c + 1) * CWORDS]
                    .rearrange("f (s w x) -> f s w x", w=2 * CBLOCKS, x=1)
                )
                nc.vector.tensor_max(
                    o_v1, s_v[:, :, :, 60:61], s_v[:, :, :, 61:62]
                )

            # --- per-quarter store (bias already applied per chunk) ---
            if c % 8 == 7:
                q = c // 8
                nc.sync.dma_start(
                    out_d.ap()[:, q * 512:(q + 1) * 512],
                    obuf[:, q * 512:(q + 1) * 512],
                )

    nc.compile()
    return nc


def host_prep(char_ids, emb_table, conv_w, conv_b, num_devices=NCORES):
    """Build per-core input maps from full inputs."""
    char_ids = np.asarray(char_ids)
    emb_table = np.asarray(emb_table, dtype=np.float32)
    conv_w = np.asarray(conv_w, dtype=np.float32)
    conv_b = np.asarray(conv_b, dtype=np.float32)

    # pair table: ptab[v1*V+v2] = [table[v1] | table[v2]], bf16
    tab_bf = emb_table.astype(ml_dtypes.bfloat16)
    ptab = np.empty((V * V, 2 * E), dtype=ml_dtypes.bfloat16)
    ptab[:, :E] = np.repeat(tab_bf, V, axis=0)
    ptab[:, E:] = np.tile(tab_bf, (V, 1))

    # stationary weights: wmat[32s+j, 128*(3h+k) + f] = conv_w[f, 2j+h, k]
    wmat = np.zeros((128, 6 * 128), dtype=np.float32)
    for h in range(2):
        for k in range(3):
            hk = 3 * h + k
            w_pf = conv_w[:, h::2, k].T  # [32 j, 128 f]
            wmat[:, 128 * hk:128 * (hk + 1)] = np.tile(w_pf, (4, 1))
    wmat = wmat.astype(ml_dtypes.bfloat16)

    bias = conv_b.reshape(128, 1)

    ids_flat = char_ids.reshape(-1, C).astype(np.int32)  # [16384 words, 32]

    in_maps = []
    for jcore in range(num_devices):
        ids_core = ids_flat[jcore * WORDS:(jcore + 1) * WORDS]  # [2048, 32]
        # pair idx, ordered i = gc*2048 + b*128 + s*32 + j with wp = 64gc+4b+s
        idv = ids_core.reshape(WORDS // 2, 2, C)  # [wp, 2, j]
        pidx = idv[:, 0, :] * V + idv[:, 1, :]  # [1024 wp, 32 j]
        # wp = 64gc + 4b + s -> order (gc, b, s, j)
        pidx = pidx.reshape(NGCHUNKS, GBLOCKS, 4, C)  # [gc, b, s, j]
        flat = pidx.reshape(-1).astype(np.int16)  # i-order
        ncols = flat.size // 16
        wrapped = flat.reshape(ncols, 16).T.copy()  # [16, ncols]
        idx = np.tile(wrapped, (8, 1))  # replicate to 128 partitions
        in_maps.append(
            {
                "idx": np.ascontiguousarray(idx),
                "tab": ptab,
                "wmat": wmat,
                "bias": bias,
            }
        )
    return in_maps


def _ensure_ntff_hook():
    """The agent image's antenv lacks axon_hooks; shim it and install the
    ctypes NTFF profiling hook so trace=True yields HW exec times."""
    import types

    if "antenv.axon_hooks" in sys.modules:
        return
    mod = types.ModuleType("antenv.axon_hooks")
    _hook = [None]
    mod.get_axon_ntff_profile_hook = lambda: _hook[0]
    mod.set_axon_ntff_profile_hook = lambda h: _hook.__setitem__(0, h)
    sys.modules["antenv.axon_hooks"] = mod
    try:
        import antenv

        antenv.axon_hooks = mod
        from trn_agent_boot.trn_boot import _ntff_profile_via_ctypes

        hook = _ntff_profile_via_ctypes("/opt/axon/libaxon_pjrt.so")
        mod.set_axon_ntff_profile_hook(hook)
    except Exception as e:  # degrade to no-trace
        print(f"ntff hook install failed: {e}", file=sys.stderr)


_NC_CACHE = {}


def _get_nc():
    if "nc" not in _NC_CACHE:
        _NC_CACHE["nc"] = build_kernel()
    return _NC_CACHE["nc"]


def unscramble_out(raw):
    """[128 f, 2048 cols], col = 64c+16s+2b+c2 -> [word, 128] word-major.

    word = 64c + 8b + 2s + c2."""
    o = raw.reshape(128, NCCHUNKS, 4, CBLOCKS, 2)  # [f, c, s, b, c2]
    o = o.transpose(1, 3, 2, 4, 0)  # [c, b, s, c2, f]
    # word = 64c + 8b + 2s + c2 -> index order (c, b, s, c2)
    return np.ascontiguousarray(o.reshape(WORDS, 128))


def kernel(char_ids, emb_table, conv_w, conv_b, trace=False):
    if trace:
        _ensure_ntff_hook()
    nc = _get_nc()
    in_maps = host_prep(char_ids, emb_table, conv_w, conv_b)
    res = run_bass_kernel_spmd(
        nc, in_maps, core_ids=list(range(NCORES)), trace=trace
    )
    outs = [unscramble_out(res.results[j]["out"]) for j in range(NCORES)]
    full = np.concatenate(outs, axis=0).reshape(B, S, F).astype(np.float32)
    if trace:
        return full, res
    return full


# revision 26
# speedup vs baseline: 1.0191x; 1.0034x over previous
"""CharEmb kernel for Trainium2 (8 NeuronCores, batch-sharded).

Computation (per word of 32 chars):
  emb = table[ids]                  # [32 chars, 64] gathered
  x[i, j] = emb[i//2, 32*(i%2)+j]   # raw-buffer reshape [64, 32]
  y[f, t] = sum_{i,k} x[i, t+k] * w[f, i, k]   (valid conv, K=3)
  out[f] = max_t y[f, t] + b[f]

Key trick vs naive: a PAIR embedding table pair_tab[v1*101+v2] =
[table[v1] | table[v2]] (128 bf16 = 256B rows) lets one gather
descriptor fetch the char-j rows of TWO adjacent words, halving the
Q7 SWDGE descriptor-generation work (the dominant cost).  Gathers are
issued round-robin on the 4 SWDGE queues so 4 Q7 core-pairs generate
descriptors concurrently.

Device mapping per core (2048 words = 1024 word-pairs, 32768 pair-gathers):
  - per 64-word chunk c: one 1024-idx gather (queue c%4); pair-idx
    i -> partition 32s+j holds [emb(word 2wp) | emb(word 2wp+1)] where
    wp = 32c + 4b + s, j = char, b = i//128 block.
  - conv: 6 accumulating K=32 matmuls per slot s, row-tiled via
    tile_position; rhs free dims (b, c2, t) pick word = 2wp + c2 and
    the (h,k) shifted window.
  - maxpool over t: 1/4 of chunks do a direct DVE tensor_reduce(max)
    from PSUM + a small ACT bias add; the rest go ACT (PSUM->SBUF bf16
    copy with the bias folded into the ACTIVATE affine, legal since
    max(y+b) == max(y)+b) then a DVE 2x-mode pairwise-max tree.
"""

import sys
from contextlib import ExitStack

import numpy as np
import ml_dtypes

if "/opt/trn_rl_repo" not in sys.path:
    sys.path.insert(0, "/opt/trn_rl_repo")

import concourse.bass as bass
import concourse.tile as tile
from concourse import bacc, mybir
from concourse.bass_utils import run_bass_kernel_spmd

# Problem constants (hardcoded per spec)
B, S, C = 32, 512, 32
V, E = 101, 64
F, K = 128, 3
T = C - K + 1  # 30 valid conv positions
NCORES = 8
WORDS = (B * S) // NCORES  # 2048 words per core

CWORDS = 64  # words per chunk (one gather + one PSUM tile)
NCCHUNKS = WORDS // CWORDS  # 32
GIDX = CWORDS // 2 * 32  # 1024 pair-indices per chunk
CBLOCKS = GIDX // 128  # 8 blocks of 128 idx per chunk
IDX_COLS = GIDX // 16  # 64 idx columns per chunk
NGCHUNKS = NCCHUNKS  # alias (gather chunk == compute chunk)
GBLOCKS = CBLOCKS

f32 = mybir.dt.float32
bf16 = mybir.dt.bfloat16
i16 = mybir.dt.int16


def build_kernel(num_devices=NCORES):
    nc = bacc.Bacc(
        "TRN2",
        target_bir_lowering=False,
        debug=False,
        enable_asserts=True,
        num_devices=num_devices,
        num_swdge_queues=4,
        dynamic_dma_scratch_size=65536,
    )

    idx_d = nc.dram_tensor("idx", [128, NCCHUNKS * IDX_COLS], i16, kind="ExternalInput")
    tab_d = nc.dram_tensor("tab", [V * V, 2 * E], bf16, kind="ExternalInput")
    w_d = nc.dram_tensor("wmat", [128, 6 * 128], bf16, kind="ExternalInput")
    b_d = nc.dram_tensor("bias", [128, 1], f32, kind="ExternalInput")
    # f-major output: out[f, col]; col = 64c + 16s + 2b + c2
    out_d = nc.dram_tensor("out", [128, WORDS], f32, kind="ExternalOutput")

    IDX_SLICES = 8
    CH_PER_SLICE = NCCHUNKS // IDX_SLICES
    NEG = -1.0e30

    with tile.TileContext(nc) as tc, ExitStack() as ctx:
        const_pool = ctx.enter_context(tc.tile_pool(name="const", bufs=1))
        g_pool = ctx.enter_context(tc.tile_pool(name="gath", bufs=12))
        p_pool = ctx.enter_context(tc.tile_pool(name="psum", bufs=2, space="PSUM"))

        idx_sb = const_pool.tile([128, NCCHUNKS * IDX_COLS], i16)
        w_sb = const_pool.tile([128, 6 * 128], bf16)
        b_sb = const_pool.tile([128, 1], f32)
        obuf = const_pool.tile([128, WORDS], f32)
        # ping-pong scratch for the ACT-copy + DVE max-tree path:
        # per (s, w) 64 cols: [0:30 y | 30:32 pad | 32:48 l1 | 48:56 l2
        #                      | 56:60 l3 | 60:62 l4]
        sbt_tiles = [
            const_pool.tile([128, 4 * 16 * 64], bf16, name=f"sbt{i}")
            for i in range(3)
        ]

        # idx DMA in slices so the first gather doesn't wait for all of it
        scols = CH_PER_SLICE * IDX_COLS
        for d in range(IDX_SLICES):
            nc.sync.dma_start(
                idx_sb[:, d * scols:(d + 1) * scols],
                idx_d.ap()[:, d * scols:(d + 1) * scols],
            )
        nc.sync.dma_start(w_sb[:], w_d.ap())
        nc.sync.dma_start(b_sb[:], b_d.ap())
        for t_ in sbt_tiles:
            # only the 2 pad cols per (s, w) group need -inf; the rest is
            # overwritten every chunk
            pad_v = t_[:].rearrange("f (s w x) -> f s w x", w=16, x=64)
            nc.vector.memset(pad_v[:, :, :, 30:32], NEG)

        nidx_reg = nc.gpsimd.to_reg(GIDX)

        for c in range(NCCHUNKS):
            g = g_pool.tile([128, CBLOCKS * 2 * E], bf16)
            nc.gpsimd.dma_gather(
                out_ap=g[:].rearrange("p (b e) -> p b e", e=2 * E),
                in_ap=tab_d.ap(),
                idxs_ap=idx_sb[:, c * IDX_COLS:(c + 1) * IDX_COLS],
                num_idxs=GIDX,
                num_idxs_reg=nidx_reg,
                elem_size=2 * E,
                single_packet=False,
                queue_num=c % 4,
            )

            # g viewed as [p, b(8), c2(2), e(64)]
            g_r = g[:].rearrange("p (b c2 e) -> p b c2 e", c2=2, e=E)

            # --- conv: 6 accumulating matmuls x 4 row-tiled slots ---
            p = p_pool.tile([128, 4 * 512], f32)
            for hk in range(6):
                h, k = divmod(hk, 3)
                j0 = 32 * h + k
                for s in range(4):
                    out_ap = (
                        p[:, 512 * s:512 * s + CBLOCKS * 2 * T]
                        .rearrange("f (b c2 t) -> f b c2 t", c2=2, t=T)
                    )
                    rhs = g_r[32 * s:32 * s + 32, :, :, j0:j0 + T]
                    lhsT = w_sb[32 * s:32 * s + 32, 128 * hk:128 * hk + 128]
                    nc.tensor.matmul(
                        out_ap,
                        lhsT,
                        rhs,
                        start=(hk == 0),
                        stop=(hk == 5),
                        tile_position=(32 * s, 0),
                        skip_group_check=True,
                    )

            # --- maxpool over t (per word) ---
            p_v = (
                p[:].rearrange("f (s x) -> f s x", x=512)[:, :, 0:CBLOCKS * 2 * T]
                .rearrange("f s (w t) -> f s w t", t=T)
            )
            o_v = (
                obuf[:, c * CWORDS:(c + 1) * CWORDS]
                .rearrange("f (s w) -> f s w", w=2 * CBLOCKS)
            )
            if c % 4 == 0 or c >= NCCHUNKS - 3:
                # direct DVE reduce from PSUM (also for the tail chunks:
                # shorter drain latency than the ACT-copy + tree path),
                # then a tiny per-chunk bias add on ACT
                nc.vector.tensor_reduce(
                    o_v, p_v, axis=mybir.AxisListType.X, op=mybir.AluOpType.max
                )
                seg = obuf[:, c * CWORDS:(c + 1) * CWORDS]
                nc.scalar.add(seg, seg, b_sb[:, 0:1])
            else:
                # ACT copies PSUM->SBUF bf16 with the bias folded in
                # (max(y+b) == max(y)+b); DVE does a 2x-mode max tree
                sbt = sbt_tiles[c % 4 - 1]
                s_v = sbt[:].rearrange("f (s w x) -> f s w x", w=16, x=64)
                nc.scalar.activation(
                    s_v[:, :, :, 0:30], p_v,
                    mybir.ActivationFunctionType.Identity, bias=b_sb[:, 0:1],
                )
                nc.vector.tensor_max(
                    s_v[:, :, :, 32:48], s_v[:, :, :, 0:16], s_v[:, :, :, 16:32]
                )
                nc.vector.tensor_max(
                    s_v[:, :, :, 48:56], s_v[:, :, :, 32:40], s_v[:, :, :, 40:48]
                )
                nc.vector.tensor_max(
                    s_v[:, :, :, 56:60], s_v[:, :, :, 48:52], s_v[:, :, :, 52:56]
                )
                nc.vector.tensor_max(
                    s_v[:, :, :, 60:62], s_v[:, :, :, 56:58], s_v[:, :, :, 58:60]
                )
                o_v1 = (
                    obuf[:, c * CWORDS:(c + 1) * CWORDS]
                    .rearrange("f (s w x) -> f s w x", w=2 * CBLOCKS, x=1)
                )
                nc.vector.tensor_max(
                    o_v1, s_v[:, :, :, 60:61], s_v[:, :, :, 61:62]
                )

            # --- per-quarter store (bias already applied per chunk) ---
            if c % 8 == 7:
                q = c // 8
                nc.sync.dma_start(
                    out_d.ap()[:, q * 512:(q + 1) * 512],
                    obuf[:, q * 512:(q + 1) * 512],
                )

    nc.compile()
    return nc


def host_prep(char_ids, emb_table, conv_w, conv_b, num_devices=NCORES):
    """Build per-core input maps from full inputs."""
    char_ids = np.asarray(char_ids)
    emb_table = np.asarray(emb_table, dtype=np.float32)
    conv_w = np.asarray(conv_w, dtype=np.float32)
    conv_b = np.asarray(conv_b, dtype=np.float32)

    # pair table: ptab[v1*V+v2] = [table[v1] | table[v2]], bf16
    tab_bf = emb_table.astype(ml_dtypes.bfloat16)
    ptab = np.empty((V * V, 2 * E), dtype=ml_dtypes.bfloat16)
    ptab[:, :E] = np.repeat(tab_bf, V, axis=0)
    ptab[:, E:] = np.tile(tab_bf, (V, 1))

    # stationary weights: wmat[32s+j, 128*(3h+k) + f] = conv_w[f, 2j+h, k]
    wmat = np.zeros((128, 6 * 128), dtype=np.float32)
    for h in range(2):
        for k in range(3):
            hk = 3 * h + k
            w_pf = conv_w[:, h::2, k].T  # [32 j, 128 f]
            wmat[:, 128 * hk:128 * (hk + 1)] = np.tile(w_pf, (4, 1))
    wmat = wmat.astype(ml_dtypes.bfloat16)

    bias = conv_b.reshape(128, 1)

    ids_flat = char_ids.reshape(-1, C).astype(np.int32)  # [16384 words, 32]

    in_maps = []
    for jcore in range(num_devices):
        ids_core = ids_flat[jcore * WORDS:(jcore + 1) * WORDS]  # [2048, 32]
        # pair idx, ordered i = gc*2048 + b*128 + s*32 + j with wp = 64gc+4b+s
        idv = ids_core.reshape(WORDS // 2, 2, C)  # [wp, 2, j]
        pidx = idv[:, 0, :] * V + idv[:, 1, :]  # [1024 wp, 32 j]
        # wp = 64gc + 4b + s -> order (gc, b, s, j)
        pidx = pidx.reshape(NGCHUNKS, GBLOCKS, 4, C)  # [gc, b, s, j]
        flat = pidx.reshape(-1).astype(np.int16)  # i-order
        ncols = flat.size // 16
        wrapped = flat.reshape(ncols, 16).T.copy()  # [16, ncols]
        idx = np.tile(wrapped, (8, 1))  # replicate to 128 partitions
        in_maps.append(
            {
                "idx": np.ascontiguousarray(idx),
                "tab": ptab,
                "wmat": wmat,
                "bias": bias,
            }
        )
    return in_maps


def _ensure_ntff_hook():
    """The agent image's antenv lacks axon_hooks; shim it and install the
    ctypes NTFF profiling hook so trace=True yields HW exec times."""
    import types

    if "antenv.axon_hooks" in sys.modules:
        return
    mod = types.ModuleType("antenv.axon_hooks")
    _hook = [None]
    mod.get_axon_ntff_profile_hook = lambda: _hook[0]
    mod.set_axon_ntff_profile_hook = lambda h: _hook.__setitem__(0, h)
    sys.modules["antenv.axon_hooks"] = mod
    try:
        import antenv

        antenv.axon_hooks = mod
        from trn_agent_boot.trn_boot import _ntff_profile_via_ctypes

        hook = _ntff_profile_via_ctypes("/opt/axon/libaxon_pjrt.so")
        mod.set_axon_ntff_profile_hook(hook)
    except Exception as e:  # degrade to no-trace
        print(f"ntff hook install failed: {e}", file=sys.stderr)


_NC_CACHE = {}


def _get_nc():
    if "nc" not in _NC_CACHE:
        _NC_CACHE["nc"] = build_kernel()
    return _NC_CACHE["nc"]


def unscramble_out(raw):
    """[128 f, 2048 cols], col = 64c+16s+2b+c2 -> [word, 128] word-major.

    word = 64c + 8b + 2s + c2."""
    o = raw.reshape(128, NCCHUNKS, 4, CBLOCKS, 2)  # [f, c, s, b, c2]
    o = o.transpose(1, 3, 2, 4, 0)  # [c, b, s, c2, f]
    # word = 64c + 8b + 2s + c2 -> index order (c, b, s, c2)
    return np.ascontiguousarray(o.reshape(WORDS, 128))


def kernel(char_ids, emb_table, conv_w, conv_b, trace=False):
    if trace:
        _ensure_ntff_hook()
    nc = _get_nc()
    in_maps = host_prep(char_ids, emb_table, conv_w, conv_b)
    res = run_bass_kernel_spmd(
        nc, in_maps, core_ids=list(range(NCORES)), trace=trace
    )
    outs = [unscramble_out(res.results[j]["out"]) for j in range(NCORES)]
    full = np.concatenate(outs, axis=0).reshape(B, S, F).astype(np.float32)
    if trace:
        return full, res
    return full


# revision 27
# speedup vs baseline: 1.1706x; 1.1486x over previous
"""CharEmb kernel for Trainium2 (8 NeuronCores, batch-sharded).

Computation (per word of 32 chars):
  emb = table[ids]                  # [32 chars, 64] gathered
  x[i, j] = emb[i//2, 32*(i%2)+j]   # raw-buffer reshape [64, 32]
  y[f, t] = sum_{i,k} x[i, t+k] * w[f, i, k]   (valid conv, K=3)
  out[f] = max_t y[f, t] + b[f]

Key trick vs naive: a PAIR embedding table pair_tab[v1*101+v2] =
[table[v1] | table[v2]] (128 bf16 = 256B rows) lets one gather
descriptor fetch the char-j rows of TWO adjacent words, halving the
Q7 SWDGE descriptor-generation work (the dominant cost).  Gathers are
issued round-robin on the 4 SWDGE queues so 4 Q7 core-pairs generate
descriptors concurrently.

Device mapping per core (2048 words = 1024 word-pairs, 32768 pair-gathers):
  - per 64-word chunk c: one 1024-idx gather (queue c%4); pair-idx
    i -> partition 32s+j holds [emb(word 2wp) | emb(word 2wp+1)] where
    wp = 32c + 4b + s, j = char, b = i//128 block.
  - conv: 6 accumulating K=32 matmuls per slot s, row-tiled via
    tile_position; rhs free dims (b, c2, t) pick word = 2wp + c2 and
    the (h,k) shifted window.
  - maxpool over t: 1/4 of chunks do a direct DVE tensor_reduce(max)
    from PSUM + a small ACT bias add; the rest go ACT (PSUM->SBUF bf16
    copy with the bias folded into the ACTIVATE affine, legal since
    max(y+b) == max(y)+b) then a DVE 2x-mode pairwise-max tree.
"""

import sys
from contextlib import ExitStack

import numpy as np
import ml_dtypes

if "/opt/trn_rl_repo" not in sys.path:
    sys.path.insert(0, "/opt/trn_rl_repo")

import concourse.bass as bass
import concourse.tile as tile
from concourse import bacc, mybir
from concourse.bass_utils import run_bass_kernel_spmd

# Problem constants (hardcoded per spec)
B, S, C = 32, 512, 32
V, E = 101, 64
F, K = 128, 3
T = C - K + 1  # 30 valid conv positions
NCORES = 8
WORDS = (B * S) // NCORES  # 2048 words per core

CWORDS = 64  # words per chunk (one gather + one PSUM tile)
NCCHUNKS = WORDS // CWORDS  # 32
GIDX = CWORDS // 2 * 32  # 1024 pair-indices per chunk
CBLOCKS = GIDX // 128  # 8 blocks of 128 idx per chunk
IDX_COLS = GIDX // 16  # 64 idx columns per chunk
NGCHUNKS = NCCHUNKS  # alias (gather chunk == compute chunk)
GBLOCKS = CBLOCKS

f32 = mybir.dt.float32
bf16 = mybir.dt.bfloat16
i16 = mybir.dt.int16


def build_kernel(num_devices=NCORES):
    nc = bacc.Bacc(
        "TRN2",
        target_bir_lowering=False,
        debug=False,
        enable_asserts=True,
        num_devices=num_devices,
        num_swdge_queues=4,
        dynamic_dma_scratch_size=65536,
    )

    idx_d = nc.dram_tensor("idx", [128, NCCHUNKS * IDX_COLS], i16, kind="ExternalInput")
    tab_d = nc.dram_tensor("tab", [V * V, 2 * E], bf16, kind="ExternalInput")
    w_d = nc.dram_tensor("wmat", [128, 6 * 128], bf16, kind="ExternalInput")
    b_d = nc.dram_tensor("bias", [128, 1], f32, kind="ExternalInput")
    # f-major output: out[f, col]; col = 64c + 16s + 2b + c2
    out_d = nc.dram_tensor("out", [128, WORDS], f32, kind="ExternalOutput")

    IDX_SLICES = 8
    CH_PER_SLICE = NCCHUNKS // IDX_SLICES
    NEG = -1.0e30

    with tile.TileContext(nc) as tc, ExitStack() as ctx:
        const_pool = ctx.enter_context(tc.tile_pool(name="const", bufs=1))
        g_pool = ctx.enter_context(tc.tile_pool(name="gath", bufs=12))
        p_pool = ctx.enter_context(tc.tile_pool(name="psum", bufs=2, space="PSUM"))

        idx_sb = const_pool.tile([128, NCCHUNKS * IDX_COLS], i16)
        w_sb = const_pool.tile([128, 6 * 128], bf16)
        b_sb = const_pool.tile([128, 1], f32)
        obuf = const_pool.tile([128, WORDS], f32)
        # ping-pong scratch for the ACT-copy + DVE max-tree path:
        # per (s, w) 64 cols: [0:30 y | 30:32 pad | 32:48 l1 | 48:56 l2
        #                      | 56:60 l3 | 60:62 l4]
        sbt_tiles = [
            const_pool.tile([128, 4 * 16 * 64], bf16, name=f"sbt{i}")
            for i in range(3)
        ]

        # idx DMA in slices so the first gather doesn't wait for all of it
        scols = CH_PER_SLICE * IDX_COLS
        for d in range(IDX_SLICES):
            nc.sync.dma_start(
                idx_sb[:, d * scols:(d + 1) * scols],
                idx_d.ap()[:, d * scols:(d + 1) * scols],
            )
        nc.sync.dma_start(w_sb[:], w_d.ap())
        nc.sync.dma_start(b_sb[:], b_d.ap())
        for t_ in sbt_tiles:
            # only the 2 pad cols per (s, w) group need -inf; the rest is
            # overwritten every chunk
            pad_v = t_[:].rearrange("f (s w x) -> f s w x", w=16, x=64)
            nc.vector.memset(pad_v[:, :, :, 30:32], NEG)

        nidx_reg = nc.gpsimd.to_reg(GIDX)

        for c in range(NCCHUNKS):
            g = g_pool.tile([128, CBLOCKS * 2 * E], bf16)
            nc.gpsimd.dma_gather(
                out_ap=g[:].rearrange("p (b e) -> p b e", e=2 * E),
                in_ap=tab_d.ap(),
                idxs_ap=idx_sb[:, c * IDX_COLS:(c + 1) * IDX_COLS],
                num_idxs=GIDX,
                num_idxs_reg=nidx_reg,
                elem_size=2 * E,
                single_packet=False,
                queue_num=c % 4,
            )

            # g viewed as [p, b(8), c2(2), e(64)]
            g_r = g[:].rearrange("p (b c2 e) -> p b c2 e", c2=2, e=E)

            # --- conv: 6 accumulating matmuls x 4 row-tiled slots ---
            p = p_pool.tile([128, 4 * 512], f32)
            for hk in range(6):
                h, k = divmod(hk, 3)
                j0 = 32 * h + k
                for s in range(4):
                    out_ap = (
                        p[:, 512 * s:512 * s + CBLOCKS * 2 * T]
                        .rearrange("f (b c2 t) -> f b c2 t", c2=2, t=T)
                    )
                    rhs = g_r[32 * s:32 * s + 32, :, :, j0:j0 + T]
                    lhsT = w_sb[32 * s:32 * s + 32, 128 * hk:128 * hk + 128]
                    nc.tensor.matmul(
                        out_ap,
                        lhsT,
                        rhs,
                        start=(hk == 0),
                        stop=(hk == 5),
                        tile_position=(32 * s, 0),
                        skip_group_check=True,
                    )

            # --- maxpool over t (per word) ---
            p_v = (
                p[:].rearrange("f (s x) -> f s x", x=512)[:, :, 0:CBLOCKS * 2 * T]
                .rearrange("f s (w t) -> f s w t", t=T)
            )
            o_v = (
                obuf[:, c * CWORDS:(c + 1) * CWORDS]
                .rearrange("f (s w) -> f s w", w=2 * CBLOCKS)
            )
            if True:  # experiment: all-direct reduce (1-port DVE reads)
                # direct DVE reduce from PSUM (also for the tail chunks:
                # shorter drain latency than the ACT-copy + tree path),
                # then a tiny per-chunk bias add on ACT
                nc.vector.tensor_reduce(
                    o_v, p_v, axis=mybir.AxisListType.X, op=mybir.AluOpType.max
                )
                seg = obuf[:, c * CWORDS:(c + 1) * CWORDS]
                nc.scalar.add(seg, seg, b_sb[:, 0:1])
            else:
                # ACT copies PSUM->SBUF bf16 with the bias folded in
                # (max(y+b) == max(y)+b); DVE does a 2x-mode max tree
                sbt = sbt_tiles[c % 4 - 1]
                s_v = sbt[:].rearrange("f (s w x) -> f s w x", w=16, x=64)
                nc.scalar.activation(
                    s_v[:, :, :, 0:30], p_v,
                    mybir.ActivationFunctionType.Identity, bias=b_sb[:, 0:1],
                )
                nc.vector.tensor_max(
                    s_v[:, :, :, 32:48], s_v[:, :, :, 0:16], s_v[:, :, :, 16:32]
                )
                nc.vector.tensor_max(
                    s_v[:, :, :, 48:56], s_v[:, :, :, 32:40], s_v[:, :, :, 40:48]
                )
                nc.vector.tensor_max(
                    s_v[:, :, :, 56:60], s_v[:, :, :, 48:52], s_v[:, :, :, 52:56]
                )
                nc.vector.tensor_max(
                    s_v[:, :, :, 60:62], s_v[:, :, :, 56:58], s_v[:, :, :, 58:60]
                )
                o_v1 = (
                    obuf[:, c * CWORDS:(c + 1) * CWORDS]
                    .rearrange("f (s w x) -> f s w x", w=2 * CBLOCKS, x=1)
                )
                nc.vector.tensor_max(
                    o_v1, s_v[:, :, :, 60:61], s_v[:, :, :, 61:62]
                )

            # --- per-quarter store (bias already applied per chunk) ---
            if c % 8 == 7:
                q = c // 8
                nc.sync.dma_start(
                    out_d.ap()[:, q * 512:(q + 1) * 512],
                    obuf[:, q * 512:(q + 1) * 512],
                )

    nc.compile()
    return nc


def host_prep(char_ids, emb_table, conv_w, conv_b, num_devices=NCORES):
    """Build per-core input maps from full inputs."""
    char_ids = np.asarray(char_ids)
    emb_table = np.asarray(emb_table, dtype=np.float32)
    conv_w = np.asarray(conv_w, dtype=np.float32)
    conv_b = np.asarray(conv_b, dtype=np.float32)

    # pair table: ptab[v1*V+v2] = [table[v1] | table[v2]], bf16
    tab_bf = emb_table.astype(ml_dtypes.bfloat16)
    ptab = np.empty((V * V, 2 * E), dtype=ml_dtypes.bfloat16)
    ptab[:, :E] = np.repeat(tab_bf, V, axis=0)
    ptab[:, E:] = np.tile(tab_bf, (V, 1))

    # stationary weights: wmat[32s+j, 128*(3h+k) + f] = conv_w[f, 2j+h, k]
    wmat = np.zeros((128, 6 * 128), dtype=np.float32)
    for h in range(2):
        for k in range(3):
            hk = 3 * h + k
            w_pf = conv_w[:, h::2, k].T  # [32 j, 128 f]
            wmat[:, 128 * hk:128 * (hk + 1)] = np.tile(w_pf, (4, 1))
    wmat = wmat.astype(ml_dtypes.bfloat16)

    bias = conv_b.reshape(128, 1)

    ids_flat = char_ids.reshape(-1, C).astype(np.int32)  # [16384 words, 32]

    in_maps = []
    for jcore in range(num_devices):
        ids_core = ids_flat[jcore * WORDS:(jcore + 1) * WORDS]  # [2048, 32]
        # pair idx, ordered i = gc*2048 + b*128 + s*32 + j with wp = 64gc+4b+s
        idv = ids_core.reshape(WORDS // 2, 2, C)  # [wp, 2, j]
        pidx = idv[:, 0, :] * V + idv[:, 1, :]  # [1024 wp, 32 j]
        # wp = 64gc + 4b + s -> order (gc, b, s, j)
        pidx = pidx.reshape(NGCHUNKS, GBLOCKS, 4, C)  # [gc, b, s, j]
        flat = pidx.reshape(-1).astype(np.int16)  # i-order
        ncols = flat.size // 16
        wrapped = flat.reshape(ncols, 16).T.copy()  # [16, ncols]
        idx = np.tile(wrapped, (8, 1))  # replicate to 128 partitions
        in_maps.append(
            {
                "idx": np.ascontiguousarray(idx),
                "tab": ptab,
                "wmat": wmat,
                "bias": bias,
            }
        )
    return in_maps


def _ensure_ntff_hook():
    """The agent image's antenv lacks axon_hooks; shim it and install the
    ctypes NTFF profiling hook so trace=True yields HW exec times."""
    import types

    if "antenv.axon_hooks" in sys.modules:
        return
    mod = types.ModuleType("antenv.axon_hooks")
    _hook = [None]
    mod.get_axon_ntff_profile_hook = lambda: _hook[0]
    mod.set_axon_ntff_profile_hook = lambda h: _hook.__setitem__(0, h)
    sys.modules["antenv.axon_hooks"] = mod
    try:
        import antenv

        antenv.axon_hooks = mod
        from trn_agent_boot.trn_boot import _ntff_profile_via_ctypes

        hook = _ntff_profile_via_ctypes("/opt/axon/libaxon_pjrt.so")
        mod.set_axon_ntff_profile_hook(hook)
    except Exception as e:  # degrade to no-trace
        print(f"ntff hook install failed: {e}", file=sys.stderr)


_NC_CACHE = {}


def _get_nc():
    if "nc" not in _NC_CACHE:
        _NC_CACHE["nc"] = build_kernel()
    return _NC_CACHE["nc"]


def unscramble_out(raw):
    """[128 f, 2048 cols], col = 64c+16s+2b+c2 -> [word, 128] word-major.

    word = 64c + 8b + 2s + c2."""
    o = raw.reshape(128, NCCHUNKS, 4, CBLOCKS, 2)  # [f, c, s, b, c2]
    o = o.transpose(1, 3, 2, 4, 0)  # [c, b, s, c2, f]
    # word = 64c + 8b + 2s + c2 -> index order (c, b, s, c2)
    return np.ascontiguousarray(o.reshape(WORDS, 128))


def kernel(char_ids, emb_table, conv_w, conv_b, trace=False):
    if trace:
        _ensure_ntff_hook()
    nc = _get_nc()
    in_maps = host_prep(char_ids, emb_table, conv_w, conv_b)
    res = run_bass_kernel_spmd(
        nc, in_maps, core_ids=list(range(NCORES)), trace=trace
    )
    outs = [unscramble_out(res.results[j]["out"]) for j in range(NCORES)]
    full = np.concatenate(outs, axis=0).reshape(B, S, F).astype(np.float32)
    if trace:
        return full, res
    return full
